# revision 1
# baseline (speedup 1.0000x reference)
"""Trainium2 Bass kernel for nn_Decoder_78237124264042.

6-layer causal decoder: V=32000, L=6, H=8, D=64, DM=512, DFF=1024, N=4, T=1024.

Sharding: 8 cores = 4 sequence-pairs. Pair {2i, 2i+1} handles sequence i with
tensor-parallel attention (4 heads per core); FFN + LayerNorm are replicated
within the pair so only one pair-AllGather (of the per-head attention outputs)
is needed per layer. Embedding gather runs on-device via dma_gather.

Compute in fp16 (PSUM accumulates fp32); softmax without max-subtraction
(logits are provably small for this model family).

Key structure choices:
- PV matmul emits TOKEN-major attention output (probabilities are the
  stationary operand), so the softmax denominator lands per-partition: the
  normalize is a [128,1] reciprocal + a per-partition-scaled scalar-engine
  copy, and the AllGather payload is already token-major (no per-head
  transposes, no 1-partition reciprocals, no partition broadcasts).
- The AllGather is split per head-pair so each collective overlaps the next
  pair's attention compute; a tiny warm-up collective at kernel start absorbs
  the first-collective rendezvous cost.
- All layer-constant bias vectors that hit the residual stream (V-projection
  bias + previous layer's folded LN2 beta) are merged into one cvec added on
  the GPSIMD engine off the critical path (broadcast-AP whole-tile ops); the
  g2/g1 residual affines also run on GPSIMD to keep the vector engine for LN.
- LN affines (g, b) are folded on the host into the downstream weight
  matrices (g2,b2 -> next layer's Wqkv; g1,b1 -> Wff; b1+bo -> residual bias)
  so the critical chain LN -> transpose -> matmul runs the bare normalize.
- Bounce-buffer writes and feature-major transposes are interleaved per
  token-half; next layer's weights are DMA'd mid-FFN2 off the critical path.
"""
import numpy as np
from contextlib import ExitStack

import concourse.bass as bass
import concourse.tile as tile
from concourse import bacc, mybir
from concourse.bass_utils import run_bass_kernel_spmd

V, L, H, D, DFF = 32000, 6, 8, 64, 1024
DM = H * D  # 512
N, T = 4, 1024
EPS = 1e-3
HC = H // 2          # heads per core
TC = T // 128        # token chunks (8)
KC = DM // 128       # dm chunks (4)
FC = DFF // 128      # dff chunks (8)
SCALE = 1.0 / np.sqrt(D)
F16 = mybir.dt.float16
F32 = mybir.dt.float32
AF = mybir.ActivationFunctionType


def _build(debug=False, no_cc=False):
    nc = bacc.Bacc(
        "TRN2",
        target_bir_lowering=False,
        debug=False,
        enable_asserts=True,
        num_devices=8,
    )

    def din(name, shape, dt=F16):
        return nc.dram_tensor(name, shape, dt, kind="ExternalInput").ap()

    emb = din("emb16", [V, DM])
    idxs = din("idxs", [128, T // 16], mybir.dt.int16)
    pos = din("pos", [128, TC, DM])
    wqk = din("wqk", [L, 128, KC, DM])       # g2[l-1]-folded for l>=1
    bqk = din("bqk", [L, 128, KC], F32)      # + b2[l-1] @ Wqk
    wv = din("wv", [L, 128, KC, HC * D])     # g2[l-1]-folded
    wff = din("wff", [L, 128, KC, DFF])      # g1[l]-folded
    bff = din("bff", [L, 128, FC], F32)      # + b1[l] @ Wff
    wo = din("wo", [L, 128, FC, DM])
    bob_rep = din("bob_rep", [L, 128, DM])   # bo[l] + b1[l], replicated
    cvec_rep = din("cvec_rep", [L, 128, DM])  # bv_full[l] + b2[l-1], replicated
    g1_rep = din("g1_rep", [L, 128, DM])
    g2_rep = din("g2_rep", [L, 128, DM])
    b2f_rep = din("b2f_rep", [128, DM])      # b2[L-1] for the final output
    diagm = din("diagm", [128, 128])         # binary keep-mask (s<=q), transposed

    out = nc.dram_tensor("out", [128, TC, DM], F32, kind="ExternalOutput").ap()

    def prb(name, tile_ap):
        if not debug:
            return
        t = nc.dram_tensor(f"prb_{name}", list(tile_ap.shape), tile_ap.dtype,
                           kind="ExternalOutput").ap()
        nc.sync.dma_start(t[:], tile_ap)

    with tile.TileContext(nc) as tc, ExitStack() as ctx:
        singles = ctx.enter_context(tc.tile_pool(name="singles", bufs=1))
        wpool = ctx.enter_context(tc.tile_pool(name="wpool", bufs=2))
        apool = ctx.enter_context(tc.tile_pool(name="apool", bufs=1))
        ypool = ctx.enter_context(tc.tile_pool(name="ypool", bufs=2))
        ppool = ctx.enter_context(tc.tile_pool(name="ppool", bufs=3))
        tpool = ctx.enter_context(tc.tile_pool(name="tpool", bufs=2))
        psum_mm = ctx.enter_context(tc.tile_pool(name="psum_mm", bufs=2, space="PSUM"))
        psum_lg = ctx.enter_context(tc.tile_pool(name="psum_lg", bufs=2, space="PSUM"))
        psum_pv = ctx.enter_context(tc.tile_pool(name="psum_pv", bufs=2, space="PSUM"))
        dram = ctx.enter_context(tc.tile_pool(name="dram", bufs=2, space="DRAM"))

        # --- persistent tiles ---
        h = singles.tile([128, TC, DM], F16)   # token-major residual master
        pos_sb = singles.tile([128, TC, DM], F16)
        idxs_sb = singles.tile([128, T // 16], mybir.dt.int16)
        diag_sb = singles.tile([128, 128], F16)
        eps_sb = singles.tile([128, 1], F32)
        nc.vector.memset(eps_sb[:], EPS)
        nc.sync.dma_start(pos_sb[:], pos[:])
        idx_load = nc.sync.dma_start(idxs_sb[:], idxs[:])
        nc.sync.dma_start(diag_sb[:], diagm[:])

        hT = singles.tile([128, KC, T], F16)
        h1T = singles.tile([128, KC, T], F16)

        # warm-up collective: absorbs the first-collective rendezvous cost
        # while the embedding gather runs.
        wua = dram.tile([128, 16], F16, tag="wua")
        wuo = dram.tile([256, 16], F16, tag="wuo")
        nc.sync.dma_start(wua[:], diag_sb[:, 0:16])
        if not no_cc:
            nc.gpsimd.collective_compute(
                "AllGather", mybir.AluOpType.bypass,
                replica_groups=[[0, 1], [2, 3], [4, 5], [6, 7]],
                ins=[wua[:].opt()], outs=[wuo[:].opt()],
            )

        # --- embedding gather: h[p, c, :] = emb16[ids[c*128+p], :] ---
        gat = nc.gpsimd.dma_gather(
            h[:], emb[:], idxs_sb[:],
            num_idxs=T, num_idxs_reg=T, elem_size=DM, elem_step=DM,
        )
        tile.add_dep_helper(gat.ins, idx_load.ins, reason="gather reads idxs_sb")
        nc.vector.tensor_add(h[:], h[:], pos_sb[:])
        prb("h0", h[:])

        # hd: token-major DRAM bounce feeding the feature-major transposes.
        hd = dram.tile([TC, 128, DM], F16, tag="hd")
        for t in range(TC):
            nc.sync.dma_start(hd[t], h[:, t, :])

        trsrc = h  # input of the next QKV (pre-affine); layer 0: h itself

        def load_weights(l, eng):
            shapes = {
                "wqk": ([128, KC, DM], F16), "wv": ([128, KC, HC * D], F16),
                "wff": ([128, KC, DFF], F16), "wo": ([128, FC, DM], F16),
                "bqk": ([128, KC], F32), "bff": ([128, FC], F32),
                "bob": ([128, DM], F16), "cvec": ([128, DM], F16),
                "g1": ([128, DM], F16), "g2": ([128, DM], F16),
            }
            w = {k: wpool.tile(s, dt, tag=k, name=f"w_{k}")
                 for k, (s, dt) in shapes.items()}
            for name, src_t in [("wqk", wqk), ("wv", wv), ("wff", wff),
                                ("wo", wo), ("bqk", bqk), ("bff", bff)]:
                eng.dma_start(w[name][:], src_t[l])
            eng.dma_start(w["bob"][:], bob_rep[l])
            eng.dma_start(w["cvec"][:], cvec_rep[l])
            eng.dma_start(w["g1"][:], g1_rep[l])
            eng.dma_start(w["g2"][:], g2_rep[l])
            return w

        def hT_transposes(dst, src_d, nh, eng):
            for k in range(KC):
                eng.dma_start_transpose(
                    dst[:, k, nh * 512:(nh + 1) * 512],
                    src_d[nh * 4:(nh + 1) * 4, :, k * 128:(k + 1) * 128]
                    .rearrange("c p d -> (c p) d"),
                )

        wnext = load_weights(0, nc.sync)
        # layer 0 feature-major input (layers >= 1 do this in the FFN2 loop)
        for nh in range(2):
            hT_transposes(hT, hd, nh, nc.sync)

        for l in range(L):
            w = wnext
            wqk_sb, wv_sb, wff_sb, wo_sb = w["wqk"], w["wv"], w["wff"], w["wo"]
            bqk_sb, bff_sb, bob_sb = w["bqk"], w["bff"], w["bob"]
            cvec_sb, g1_sb, g2_sb = w["cvec"], w["g1"], w["g2"]
            if l == 0:
                prb("hT0", hT[:])

            # --- qkT = WqkT @ h : rows [q(4 heads)|k(4 heads)], cols T ---
            qk_sb = apool.tile([128, KC, T], F16, tag="qk")
            for n in range(2):
                for m in range(4):
                    ps = psum_mm.tile([128, 512], F32, tag="mm")
                    for k in range(KC):
                        nc.tensor.matmul(
                            ps[:],
                            wqk_sb[:, k, m * 128:(m + 1) * 128],
                            hT[:, k, n * 512:(n + 1) * 512],
                            start=(k == 0), stop=(k == KC - 1),
                        )
                    dst = qk_sb[:, m, n * 512:(n + 1) * 512]
                    if m % 2 == 0:
                        nc.scalar.activation(dst, ps[:], AF.Identity,
                                             bias=bqk_sb[:, m:m + 1])
                    else:
                        nc.vector.tensor_scalar(
                            dst, ps[:], bqk_sb[:, m:m + 1], None,
                            mybir.AluOpType.add)

            # --- v (token-major, with ones column for the softmax denom) ---
            v_sb = apool.tile([128, TC, HC, D + 1], F16, tag="v")
            nc.vector.memset(v_sb[:, :, :, D:D + 1], 1.0)
            for t in range(TC):
                ps = psum_mm.tile([128, HC * D], F32, tag="mm")
                for k in range(KC):
                    nc.tensor.matmul(
                        ps[:],
                        hT[:, k, t * 128:(t + 1) * 128],
                        wv_sb[:, k, :],
                        start=(k == 0), stop=(k == KC - 1),
                    )
                nc.vector.tensor_copy(
                    v_sb[:, t, :, 0:D],
                    ps[:].rearrange("p (h d) -> p h d", h=HC),
                )

            # --- residual base (GPSIMD, off critical path):
            # x = y2*g2 + cvec   (cvec = V-bias + b2[l-1]; l=0: x = h + cvec)
            g2b = g2_sb[:].rearrange("p d -> p () d").broadcast_to([128, TC, DM])
            cvb = cvec_sb[:].rearrange("p d -> p () d").broadcast_to([128, TC, DM])
            if l > 0:
                nc.gpsimd.tensor_mul(h[:], trsrc[:], g2b)
            nc.gpsimd.tensor_add(h[:], h[:], cvb)

            # --- attention (4 local heads, by pairs), token-major output ---
            # After each head pair, its 128 output columns are AllGather-ed so
            # the collective overlaps the next pair's compute.
            a_all = apool.tile([128, TC, HC * D], F16, tag="a_all")
            a_tok = apool.tile([128, TC, DM], F16, tag="a_tok")
            for pair in range(2):
                for hh in (2 * pair, 2 * pair + 1):
                    qT = qk_sb[64 * (hh % 2):64 * (hh % 2) + 64, hh // 2, :]
                    kT = qk_sb[64 * (hh % 2):64 * (hh % 2) + 64, 2 + hh // 2, :]
                    # phase 1: pT[si] = exp(scale * K_si^T Q), diag-masked
                    pts = []
                    for si in range(TC):
                        q0 = si * 128
                        lg = psum_lg.tile([128, T], F32, tag="lg")
                        if q0 < 512:
                            nc.tensor.matmul(lg[:, q0:512], kT[:, q0:q0 + 128],
                                             qT[:, q0:512], start=True, stop=True)
                            nc.tensor.matmul(lg[:, 512:1024], kT[:, q0:q0 + 128],
                                             qT[:, 512:1024], start=True, stop=True)
                        else:
                            nc.tensor.matmul(lg[:, q0:1024], kT[:, q0:q0 + 128],
                                             qT[:, q0:1024], start=True, stop=True)
                        pT = tpool.tile([128, T], F16, tag=f"pT{si}")
                        cols = T - q0
                        nc.scalar.activation(pT[:, 0:cols], lg[:, q0:T], AF.Exp,
                                             scale=float(SCALE))
                        nc.vector.tensor_mul(pT[:, 0:128], pT[:, 0:128], diag_sb[:])
                        pts.append(pT)
                    # phase 2: token-major PV; col D is the softmax denominator
                    for qi in range(TC):
                        pv = psum_pv.tile([128, D + 1], F32, tag="pv")
                        for si in range(qi + 1):
                            off = (qi - si) * 128
                            nc.tensor.matmul(
                                pv[:],
                                pts[si][:, off:off + 128],
                                v_sb[:, si, hh, :],
                                start=(si == 0), stop=(si == qi),
                            )
                        rden = ppool.tile([128, 1], F32, tag="rden")
                        nc.vector.reciprocal(rden[:], pv[:, D:D + 1])
                        nc.scalar.activation(
                            a_all[:, qi, hh * D:(hh + 1) * D], pv[:, 0:D],
                            AF.Identity, scale=rden[:, 0:1])
                # AllGather this pair's 128 columns; assemble + accumulate
                c0, c1 = pair * 128, pair * 128 + 128
                agi = dram.tile([128, TC, 128], F16, tag=f"agi{pair}")
                ago = dram.tile([256, TC, 128], F16, tag=f"ago{pair}")
                nc.sync.dma_start(agi[:], a_all[:, :, c0:c1])
                if no_cc:
                    nc.sync.dma_start(ago[0:128], agi[:])
                    nc.sync.dma_start(ago[128:256], agi[:])
                else:
                    nc.gpsimd.collective_compute(
                        "AllGather", mybir.AluOpType.bypass,
                        replica_groups=[[0, 1], [2, 3], [4, 5], [6, 7]],
                        ins=[agi[:].opt()], outs=[ago[:].opt()],
                    )
                nc.sync.dma_start(a_tok[:, :, c0:c1], ago[0:128])
                nc.sync.dma_start(a_tok[:, :, 256 + c0:256 + c1], ago[128:256])
                nc.vector.tensor_add(
                    h[:, :, c0:c1], h[:, :, c0:c1], a_tok[:, :, c0:c1])
                nc.vector.tensor_add(
                    h[:, :, 256 + c0:256 + c1], h[:, :, 256 + c0:256 + c1],
                    a_tok[:, :, 256 + c0:256 + c1])
            if l == 0:
                prb("a0", a_all[:])

            # --- LN1 -> y1; h1d bounce; x2base on GPSIMD (per half) ---
            y1 = ypool.tile([128, TC, DM], F16, tag="y")
            h1d = dram.tile([TC, 128, DM], F16, tag="h1d")
            g1b = g1_sb[:].rearrange("p d -> p () d").broadcast_to([128, 4, DM])
            bobb = bob_sb[:].rearrange("p d -> p () d").broadcast_to([128, 4, DM])
            for half in range(2):
                t0, t1 = 4 * half, 4 * half + 4
                for t in range(t0, t1):
                    _ln_chunk(nc, ppool, h, y1, t, eps_sb)
                    nc.sync.dma_start(h1d[t], y1[:, t, :])
                hT_transposes(h1T, h1d, half, nc.sync)
                # x2base = y1*g1 + (b1+bo), off critical path
                nc.gpsimd.tensor_mul(h[:, t0:t1, :], y1[:, t0:t1, :], g1b)
                nc.gpsimd.tensor_add(h[:, t0:t1, :], h[:, t0:t1, :], bobb)

            # --- FFN: ffT = relu(Wff'T @ y1 + bff'); o = ffT.T @ Wo ---
            ff_sb = apool.tile([128, FC, T], F16, tag="ff")
            for n in range(2):
                for m in range(FC):
                    ps = psum_mm.tile([128, 512], F32, tag="mm")
                    for k in range(KC):
                        nc.tensor.matmul(
                            ps[:],
                            wff_sb[:, k, m * 128:(m + 1) * 128],
                            h1T[:, k, n * 512:(n + 1) * 512],
                            start=(k == 0), stop=(k == KC - 1),
                        )
                    dst = ff_sb[:, m, n * 512:(n + 1) * 512]
                    if m % 2 == 0:
                        nc.scalar.activation(dst, ps[:], AF.Relu,
                                             bias=bff_sb[:, m:m + 1])
                    else:
                        nc.vector.tensor_scalar(
                            dst, ps[:], bff_sb[:, m:m + 1], 0.0,
                            mybir.AluOpType.add, mybir.AluOpType.max)

            # --- FFN2 + residual + LN2; hd bounce for next layer ---
            y2 = ypool.tile([128, TC, DM], F16, tag="y")
            for t in range(TC):
                ps = psum_mm.tile([128, DM], F32, tag="mm")
                for k in range(FC):
                    nc.tensor.matmul(
                        ps[:],
                        ff_sb[:, k, t * 128:(t + 1) * 128],
                        wo_sb[:, k, :],
                        start=(k == 0), stop=(k == FC - 1),
                    )
                nc.vector.tensor_add(h[:, t, :], h[:, t, :], ps[:])
                _ln_chunk(nc, ppool, h, y2, t, eps_sb)
                if l < L - 1:
                    nc.sync.dma_start(hd[t], y2[:, t, :])
                    if t == 3 or t == 7:
                        hT_transposes(hT, hd, t // 4, nc.sync)
                    if t == 3:
                        wnext = load_weights(l + 1, nc.sync)
            trsrc = y2

        # --- output: h_final = y2*g2 + b2, cast to f32 ---
        b2f_sb = singles.tile([128, DM], F16)
        nc.sync.dma_start(b2f_sb[:], b2f_rep[:])
        ho = singles.tile([128, TC, DM], F32)
        for t in range(TC):
            nc.vector.tensor_mul(h[:, t, :], trsrc[:, t, :], g2_sb[:])
            nc.vector.tensor_add(h[:, t, :], h[:, t, :], b2f_sb[:])
            nc.scalar.copy(ho[:, t, :], h[:, t, :])
            nc.sync.dma_start(out[:, t, :], ho[:, t, :])

    nc.finalize()
    return nc


def _ln_chunk(nc, pool, x, y, t, eps_sb):
    """LayerNorm (no affine) of chunk t: y[:, t, :] = (x_t - mean)/std."""
    stats = pool.tile([128, TC, 6], F32, tag="ln_stats")
    mv = pool.tile([128, TC, 2], F32, tag="ln_mv")
    rstd = pool.tile([128, TC], F32, tag="ln_rstd")
    nc.vector.bn_stats(stats[:, t, :], x[:, t, :])
    nc.vector.bn_aggr(mv[:, t, :], stats[:, t, :])
    nc.scalar.activation(rstd[:, t:t + 1], mv[:, t, 1:2], AF.Sqrt, bias=eps_sb[:])
    nc.vector.reciprocal(rstd[:, t:t + 1], rstd[:, t:t + 1])
    nc.vector.tensor_scalar(
        y[:, t, :], x[:, t, :],
        mv[:, t, 0:1], rstd[:, t:t + 1],
        mybir.AluOpType.subtract, mybir.AluOpType.mult,
    )


_NC_CACHE = {}


def _get_nc(**kw):
    key = tuple(sorted(kw.items()))
    if key not in _NC_CACHE:
        _NC_CACHE[key] = _build(**kw)
    return _NC_CACHE[key]


def _prep_inputs(x, emb, Wqkv, bqkv, Wff, bff, Wo, bo, g1, beta1, g2, beta2):
    """Host-side sharding + LN-affine folding: build the 8 per-core maps."""
    f16 = np.float16
    f32 = np.float32
    emb16 = np.ascontiguousarray((np.asarray(emb) * np.sqrt(f32(DM))).astype(f16))

    p_ = np.arange(T, dtype=f32)[:, None]
    i_ = np.arange(DM, dtype=f32)[None, :]
    rates = 1.0 / np.power(10000.0, 2.0 * np.floor(i_ / 2.0) / DM)
    ang = p_ * rates
    even = (np.arange(DM) % 2) == 0
    pos = np.where(even[None, :], np.sin(ang), np.cos(ang)).astype(f16)
    pos_l = np.ascontiguousarray(pos.reshape(TC, 128, DM).transpose(1, 0, 2))

    Wqkv = np.asarray(Wqkv, f32)
    bqkv = np.asarray(bqkv, f32)
    Wff_ = np.asarray(Wff, f32)
    Wo_ = np.asarray(Wo, f32)
    bff_ = np.asarray(bff, f32)
    bo_ = np.asarray(bo, f32)
    g1_ = np.asarray(g1, f32)
    b1_ = np.asarray(beta1, f32)
    g2_ = np.asarray(g2, f32)
    b2_ = np.asarray(beta2, f32)

    # fold g2[l-1], b2[l-1] into layer l's QKV weights (l >= 1)
    gprev = np.ones((L, DM), f32)
    bprev = np.zeros((L, DM), f32)
    gprev[1:] = g2_[:-1]
    bprev[1:] = b2_[:-1]
    Wqkv_f = Wqkv * gprev[:, :, None]
    bqkv_f = bqkv + np.einsum("ld,ldc->lc", bprev, Wqkv)
    # fold g1[l], b1[l] into Wff
    Wff_f = Wff_ * g1_[:, :, None]
    bff_f = bff_ + np.einsum("ld,ldc->lc", b1_, Wff_)
    bob = (bo_ + b1_).astype(f16)

    Wh = Wqkv_f.reshape(L, DM, H, D, 3)
    bh = bqkv_f.reshape(L, H, D, 3)

    def dm_part(w):  # [L, DM, C] -> [L, 128, KC, C]
        Lx, dm, C = w.shape
        return np.ascontiguousarray(
            w.reshape(Lx, dm // 128, 128, C).transpose(0, 2, 1, 3))

    wff_l = dm_part(Wff_f).astype(f16)
    wo_l = dm_part(Wo_).astype(f16)
    bff_l = np.ascontiguousarray(bff_f.reshape(L, FC, 128).transpose(0, 2, 1))

    def rep(v):  # [L, DM] -> [L, 128, DM] replicated f16
        return np.ascontiguousarray(np.broadcast_to(
            np.asarray(v, f16)[:, None, :], (L, 128, DM)))

    # cvec = V-projection bias (folded) + b2[l-1]: hits the residual directly
    bv_full = bh[:, :, :, 2].reshape(L, DM)
    cvec = bv_full + bprev

    bob_l = rep(bob)
    cvec_l = rep(cvec)
    g1_l = rep(g1_)
    g2_l = rep(g2_)
    b2f = np.ascontiguousarray(
        np.broadcast_to(b2_[L - 1].astype(f16)[None, :], (128, DM)))

    s_i = np.arange(128)[:, None]
    q_i = np.arange(128)[None, :]
    diag = (s_i <= q_i).astype(f16)

    x = np.asarray(x)
    in_maps = []
    for c in range(8):
        seq, half = c // 2, c % 2
        hs = slice(half * HC, half * HC + HC)
        wq = Wh[:, :, hs, :, 0].reshape(L, DM, HC * D)
        wk = Wh[:, :, hs, :, 1].reshape(L, DM, HC * D)
        wqk_c = dm_part(np.concatenate([wq, wk], axis=2)).astype(f16)
        bq = bh[:, hs, :, 0].reshape(L, HC * D)
        bk = bh[:, hs, :, 1].reshape(L, HC * D)
        bqk_c = np.ascontiguousarray(
            np.concatenate([bq, bk], 1).reshape(L, KC, 128).transpose(0, 2, 1)
        ).astype(f32)
        wv_c = dm_part(Wh[:, :, hs, :, 2].reshape(L, DM, HC * D)).astype(f16)

        ids = np.asarray(x[seq], np.int64)
        idx_w = np.ascontiguousarray(
            np.tile(ids.reshape(T // 16, 16).T.astype(np.int16), (8, 1)))

        in_maps.append({
            "emb16": emb16, "idxs": idx_w, "pos": pos_l,
            "wqk": wqk_c, "bqk": bqk_c, "wv": wv_c,
            "wff": wff_l, "bff": bff_l, "wo": wo_l, "bob_rep": bob_l,
            "cvec_rep": cvec_l,
            "g1_rep": g1_l, "g2_rep": g2_l, "b2f_rep": b2f,
            "diagm": diag,
        })
    return in_maps


def kernel(**inputs) -> np.ndarray:
    nc = _get_nc()
    in_maps = _prep_inputs(**inputs)
    res = run_bass_kernel_spmd(nc, in_maps, core_ids=list(range(8)))
    outs = []
    for seq in range(N):
        o = res.results[2 * seq]["out"]  # [128, TC, DM], token t = c*128+p
        outs.append(o.transpose(1, 0, 2).reshape(T, DM))
    return np.stack(outs).astype(np.float32)



# revision 3
# speedup vs baseline: 1.0599x; 1.0599x over previous
"""Trainium2 Bass kernel v2 for nn_Decoder_78237124264042.

Query-split design: 8 cores = 4 sequence-pairs. Pair {2i, 2i+1} handles
sequence i; parity-0 core owns token blocks {0,3,4,7}, parity-1 {1,2,5,6}
(causally balanced). Each core runs attention for ALL 8 heads over its 4
local query blocks and keeps LN/FFN local (half-size tokens).

Cross-core data is ONE 0.5MB AllGather of the feature-major y2 (hT) per
layer, issued right after the FFN2 transposes and consumed a full
QKV+local-attention window later (~25us of cover vs ~15-20us collective
latency incl. the peer-driven inbound lag) — so it never stalls. The
rank-relative readback uses dma_gather with per-core index data (rows of
the partner's AllGather half), keeping the program SPMD-uniform. Remote
tokens' k/v are then computed locally from hT_rem (the extra PE work fits
inside the scalar-bound local-exp window), which makes the kT/v tiles
fully uniform: slots 0-3 local blocks, 4-7 remote blocks.

Causal structure (uniform across parity):
- local key step s vs local query ordinal j: attend iff s <= j, diagonal
  mask exactly at s == j (compile-time uniform).
- remote key step s vs query ordinal j: superset s <= j; the only blocks
  that differ by parity are s == j, handled by a 0/1 per-(core,s) mask
  input applied to the first 128 columns of the remote pT tile.

Head-pair packing: q is stored in qe/qo tiles with zeroed partition
halves so one kT stationary serves both heads of a pair; exp and mask
run once per (pair, step). PV output is token-major with a ones-column
denominator; normalization+residual+LN1+PE-transpose run per query chunk
(j-outer PV) so they overlap the remaining PV. All transposes are PE
transposes (identity matmul into PSUM) — no DRAM bounce anywhere.
"""
import numpy as np
from contextlib import ExitStack

import concourse.bass as bass
import concourse.tile as tile
from concourse import bacc, mybir
from concourse.bass_utils import run_bass_kernel_spmd

V, L, H, D, DFF = 32000, 6, 8, 64, 1024
DM = H * D  # 512
N, T = 4, 1024
EPS = 1e-3
NB = 4               # local 128-token blocks per core
LT = NB * 128        # local tokens = 512
KC = DM // 128       # 4
FC = DFF // 128      # 8
NP = H // 2          # head pairs = 4
SCALE = 1.0 / np.sqrt(D)
F16 = mybir.dt.float16
F32 = mybir.dt.float32
AF = mybir.ActivationFunctionType

P0 = [0, 3, 4, 7]
P1 = [1, 2, 5, 6]

HTW = KC * LT        # hT payload cols = 2048 (4096B row, 256B-aligned)


def _build(debug=False, no_cc=False):
    nc = bacc.Bacc(
        "TRN2",
        target_bir_lowering=False,
        debug=False,
        enable_asserts=True,
        num_devices=8,
    )

    def din(name, shape, dt=F16):
        return nc.dram_tensor(name, shape, dt, kind="ExternalInput").ap()

    emb = din("emb16", [V, DM])
    idxs = din("idxs", [128, LT // 16], mybir.dt.int16)
    idxr = din("idxr", [128, 128 // 16], mybir.dt.int16)
    pos = din("pos", [128, NB, DM])
    wqk = din("wqk", [L, 128, KC, 1024])      # [q(512) | k(512)], g2[l-1]-folded
    bqk = din("bqk", [L, 128, 8], F32)        # q psum m -> col m, k -> col 4+m
    wv = din("wv", [L, 128, KC, DM])          # g2[l-1]-folded
    wff = din("wff", [L, 128, KC, DFF])       # g1[l]-folded
    bff = din("bff", [L, 128, FC], F32)
    wo = din("wo", [L, 128, FC, DM])
    bob_rep = din("bob_rep", [L, 128, DM])    # bo + b1
    cvec_rep = din("cvec_rep", [L, 128, DM])  # bv_full + b2[l-1]
    g1_rep = din("g1_rep", [L, 128, DM])
    g2_rep = din("g2_rep", [L, 128, DM])
    b2f_rep = din("b2f_rep", [128, DM])
    diagm = din("diagm", [128, 128])          # keep-mask (key<=query)
    maskr = din("maskr", [128, NB])           # 0/1 remote diag-step mask
    ident = din("ident", [128, 128])

    out = nc.dram_tensor("out", [128, NB, DM], F32, kind="ExternalOutput").ap()

    def prb(name, tile_ap):
        if not debug:
            return
        t = nc.dram_tensor(f"prb_{name}", list(tile_ap.shape), tile_ap.dtype,
                           kind="ExternalOutput").ap()
        nc.sync.dma_start(t[:], tile_ap)

    with tile.TileContext(nc) as tc, ExitStack() as ctx:
        singles = ctx.enter_context(tc.tile_pool(name="singles", bufs=1))
        wpool = ctx.enter_context(tc.tile_pool(name="wpool", bufs=2))
        ypool = ctx.enter_context(tc.tile_pool(name="ypool", bufs=2))
        ppool = ctx.enter_context(tc.tile_pool(name="ppool", bufs=3))
        ptpool = ctx.enter_context(tc.tile_pool(name="ptpool", bufs=1))
        apool = ctx.enter_context(tc.tile_pool(name="apool", bufs=1))
        psum_lg = ctx.enter_context(tc.tile_pool(name="psum_lg", bufs=2, space="PSUM"))
        psum_mm = ctx.enter_context(tc.tile_pool(name="psum_mm", bufs=2, space="PSUM"))
        psum_pv = ctx.enter_context(tc.tile_pool(name="psum_pv", bufs=2, space="PSUM"))
        dram = ctx.enter_context(tc.tile_pool(name="dram", bufs=2, space="DRAM"))

        # --- persistent tiles ---
        h = singles.tile([128, NB, DM], F16)      # token-major residual (local)
        hT = singles.tile([128, KC, LT], F16)     # feature-major y2 (local)
        hTr_g = singles.tile([128, 1, HTW], F16)  # gather dst: partner's hT
        h1T = singles.tile([128, KC, LT], F16)    # feature-major y1 (FFN input)
        qe = singles.tile([128, NP, LT], F16)     # even head q; rows 64-127 zero
        qo = singles.tile([128, NP, LT], F16)     # odd head q; rows 0-63 zero
        kT = singles.tile([128, NP, 2 * NB, 128], F16)   # slots: 0-3 loc, 4-7 rem
        v_all = singles.tile([128, 2 * NB, H, D + 1], F16)
        a_sb = singles.tile([128, NB, H, D + 1], F16)
        rden = singles.tile([128, NB, H], F32)
        pos_sb = singles.tile([128, NB, DM], F16)
        idxs_sb = singles.tile([128, LT // 16], mybir.dt.int16)
        idxr_sb = singles.tile([128, 128 // 16], mybir.dt.int16)
        diag_sb = singles.tile([128, 128], F16)
        maskr_sb = singles.tile([128, NB], F16)
        ident_sb = singles.tile([128, 128], F16)
        eps_sb = singles.tile([128, 1], F32)
        ho = singles.tile([128, NB, DM], F32)

        hT_rem = hTr_g[:, 0, :].rearrange("p (a b) -> p a b", a=KC, b=LT)

        nc.vector.memset(eps_sb[:], EPS)
        nc.vector.memset(qe[64:128, :, :], 0.0)
        nc.vector.memset(qo[0:64, :, :], 0.0)
        nc.vector.memset(v_all[:, :, :, D:D + 1], 1.0)
        nc.sync.dma_start(pos_sb[:], pos[:])
        idx_load = nc.sync.dma_start(idxs_sb[:], idxs[:])
        idxr_load = nc.sync.dma_start(idxr_sb[:], idxr[:])
        nc.sync.dma_start(diag_sb[:], diagm[:])
        nc.sync.dma_start(maskr_sb[:], maskr[:])
        nc.sync.dma_start(ident_sb[:], ident[:])

        # warm-up collective: absorbs first-collective rendezvous cost.
        wua = dram.tile([128, 16], F16, tag="wua")
        wuo = dram.tile([256, 16], F16, tag="wuo")
        nc.sync.dma_start(wua[:], diag_sb[:, 0:16])
        if not no_cc:
            nc.gpsimd.collective_compute(
                "AllGather", mybir.AluOpType.bypass,
                replica_groups=[[0, 1], [2, 3], [4, 5], [6, 7]],
                ins=[wua[:].opt()], outs=[wuo[:].opt()],
            )

        # --- embedding gather for local tokens ---
        gat = nc.gpsimd.dma_gather(
            h[:], emb[:], idxs_sb[:],
            num_idxs=LT, num_idxs_reg=LT, elem_size=DM, elem_step=DM,
        )
        tile.add_dep_helper(gat.ins, idx_load.ins, reason="gather reads idxs_sb")
        nc.vector.tensor_add(h[:], h[:], pos_sb[:])
        prb("h0", h[:])

        def load_weights(l, eng):
            shapes = {
                "wqk": ([128, KC, 1024], F16), "wv": ([128, KC, DM], F16),
                "wff": ([128, KC, DFF], F16), "wo": ([128, FC, DM], F16),
                "bqk": ([128, 8], F32), "bff": ([128, FC], F32),
                "bob": ([128, DM], F16), "cvec": ([128, DM], F16),
                "g1": ([128, DM], F16), "g2": ([128, DM], F16),
            }
            w = {k: wpool.tile(s, dt, tag=k, name=f"w_{k}")
                 for k, (s, dt) in shapes.items()}
            for name, src_t in [("wqk", wqk), ("wv", wv), ("wff", wff),
                                ("wo", wo), ("bqk", bqk), ("bff", bff)]:
                eng.dma_start(w[name][:], src_t[l])
            eng.dma_start(w["bob"][:], bob_rep[l])
            eng.dma_start(w["cvec"][:], cvec_rep[l])
            eng.dma_start(w["g1"][:], g1_rep[l])
            eng.dma_start(w["g2"][:], g2_rep[l])
            return w

        def transpose_chunk(dst, src, j, eng=None):
            """PE-transpose src[:, j, :] ([128 tok, DM]) into dst[:, :, j*128:...]."""
            tr = psum_mm.tile([128, KC, 128], F16, tag="mm", name="tr")
            for k in range(KC):
                nc.tensor.transpose(
                    tr[:, k, :], src[:, j, k * 128:(k + 1) * 128], ident_sb[:])
            if eng is None:
                nc.scalar.copy(dst[:, :, j * 128:(j + 1) * 128], tr[:])
            else:
                eng.tensor_copy(dst[:, :, j * 128:(j + 1) * 128], tr[:])

        def hT_exchange():
            """AllGather local hT; partner's half lands in hTr_g via gather."""
            agi = dram.tile([128, HTW], F16, tag="agi")
            ago = dram.tile([256, HTW], F16, tag="ago")
            nc.sync.dma_start(agi[:], hT[:].rearrange("p a b -> p (a b)"))
            if no_cc:
                nc.sync.dma_start(ago[0:128], agi[:])
                nc.sync.dma_start(ago[128:256], agi[:])
            else:
                nc.gpsimd.collective_compute(
                    "AllGather", mybir.AluOpType.bypass,
                    replica_groups=[[0, 1], [2, 3], [4, 5], [6, 7]],
                    ins=[agi[:].opt()], outs=[ago[:].opt()],
                )
            return ago

        def hT_collect(ago):
            g = nc.gpsimd.dma_gather(
                hTr_g[:], ago[:], idxr_sb[:],
                num_idxs=128, num_idxs_reg=128, elem_size=HTW, elem_step=HTW,
            )
            tile.add_dep_helper(g.ins, idxr_load.ins, reason="hT gather idxr")

        # layer 0 feature-major input + first exchange
        wnext = load_weights(0, nc.sync)
        for j in range(NB):
            transpose_chunk(hT, h, j, eng=nc.vector)
        # layer-0 residual base: h += cvec[0]
        cvb0 = wnext["cvec"][:].rearrange("p d -> p () d").broadcast_to([128, NB, DM])
        nc.gpsimd.tensor_add(h[:], h[:], cvb0)
        ago_infl = hT_exchange()

        for l in range(L):
            w = wnext
            wqk_sb, wv_sb, wff_sb, wo_sb = w["wqk"], w["wv"], w["wff"], w["wo"]
            bqk_sb, bff_sb, bob_sb = w["bqk"], w["bff"], w["bob"]
            g1_sb, g2_sb = w["g1"], w["g2"]

            # collect partner's hT (issued at the end of the previous layer)
            hT_collect(ago_infl)

            # psum evacuation split across scalar+vector so the mm ring
            # frees in ~half the time
            def evac2(dst, ps, bias):
                hw = ps.shape[-1] // 2
                nc.scalar.activation(dst[:, 0:hw], ps[:, 0:hw], AF.Identity,
                                     bias=bias)
                nc.vector.tensor_scalar(dst[:, hw:], ps[:, hw:], bias,
                                        None, mybir.AluOpType.add)

            # --- local k, q, v ---
            for m in range(NP):
                ps = psum_mm.tile([128, 512], F32, tag="mm")
                for k in range(KC):
                    nc.tensor.matmul(
                        ps[:], wqk_sb[:, k, 512 + m * 128:512 + (m + 1) * 128],
                        hT[:, k, :], start=(k == 0), stop=(k == KC - 1))
                dst = kT[:, m, 0:NB, :].rearrange("p a b -> p (a b)")
                evac2(dst, ps[:], bqk_sb[:, 4 + m:5 + m])
            for m in range(NP):
                ps = psum_mm.tile([128, 512], F32, tag="mm")
                for k in range(KC):
                    nc.tensor.matmul(
                        ps[:], wqk_sb[:, k, m * 128:(m + 1) * 128],
                        hT[:, k, :], start=(k == 0), stop=(k == KC - 1))
                nc.scalar.activation(qe[0:64, m, 0:256], ps[0:64, 0:256],
                                     AF.Identity, bias=bqk_sb[0:64, m:m + 1])
                nc.vector.tensor_scalar(qe[0:64, m, 256:512], ps[0:64, 256:512],
                                        bqk_sb[0:64, m:m + 1], None,
                                        mybir.AluOpType.add)
                nc.scalar.activation(qo[64:128, m, 0:256], ps[64:128, 0:256],
                                     AF.Identity, bias=bqk_sb[64:128, m:m + 1])
                nc.vector.tensor_scalar(qo[64:128, m, 256:512],
                                        ps[64:128, 256:512],
                                        bqk_sb[64:128, m:m + 1], None,
                                        mybir.AluOpType.add)
            for j in range(NB):
                ps = psum_mm.tile([128, 512], F32, tag="mm")
                for k in range(KC):
                    nc.tensor.matmul(
                        ps[:], hT[:, k, j * 128:(j + 1) * 128], wv_sb[:, k, :],
                        start=(k == 0), stop=(k == KC - 1))
                nc.vector.tensor_copy(
                    v_all[:, j, :, 0:D],
                    ps[:].rearrange("p (h d) -> p h d", h=H))

            # --- logits + exp: local steps (kT slots 0-3) ---
            # pts[(which, p, s)] = (tile, col base of step s within it);
            # steps 2 and 3 share one psum tile + one exp.
            pts = {}

            def pt_mask(which, pt, s, base):
                if which == "l":
                    nc.vector.tensor_mul(
                        pt[:, :, base:base + 128], pt[:, :, base:base + 128],
                        diag_sb[:].rearrange("p d -> p () d")
                        .broadcast_to([128, 2, 128]))
                else:
                    nc.vector.tensor_mul(
                        pt[:, :, base:base + 128], pt[:, :, base:base + 128],
                        maskr_sb[:, s:s + 1].rearrange("p o -> p o ()")
                        .broadcast_to([128, 2, 128]))

            def logits_block(which, p, s, slot_base, kt_tile):
                C = (NB - s) * 128
                lg = psum_lg.tile([128, 2, 512], F32, tag="lg")
                nc.tensor.matmul(lg[:, 0, 0:C], kt_tile[:, p, slot_base + s, :],
                                 qe[:, p, s * 128:LT], start=True, stop=True)
                nc.tensor.matmul(lg[:, 1, 0:C], kt_tile[:, p, slot_base + s, :],
                                 qo[:, p, s * 128:LT], start=True, stop=True)
                pt = ptpool.tile([128, 2, C], F16,
                                 tag=f"pt{which}{p}{s}", name="pt")
                nc.scalar.activation(pt[:], lg[:, :, 0:C], AF.Exp,
                                     scale=float(SCALE))
                pt_mask(which, pt, s, 0)
                pts[(which, p, s)] = (pt, 0)

            def logits_block23(which, p, slot_base, kt_tile):
                lg = psum_lg.tile([128, 2, 512], F32, tag="lg")
                for hh, q_t in ((0, qe), (1, qo)):
                    nc.tensor.matmul(lg[:, hh, 0:256],
                                     kt_tile[:, p, slot_base + 2, :],
                                     q_t[:, p, 256:LT], start=True, stop=True)
                    nc.tensor.matmul(lg[:, hh, 256:384],
                                     kt_tile[:, p, slot_base + 3, :],
                                     q_t[:, p, 384:LT], start=True, stop=True)
                pt = ptpool.tile([128, 2, 384], F16,
                                 tag=f"pt{which}{p}23", name="pt")
                nc.scalar.activation(pt[:], lg[:, :, 0:384], AF.Exp,
                                     scale=float(SCALE))
                pt_mask(which, pt, 2, 0)
                pt_mask(which, pt, 3, 256)
                pts[(which, p, 2)] = (pt, 0)
                pts[(which, p, 3)] = (pt, 256)

            def logits_pair(which, p, slot_base, kt_tile):
                logits_block(which, p, 0, slot_base, kt_tile)
                logits_block(which, p, 1, slot_base, kt_tile)
                logits_block23(which, p, slot_base, kt_tile)

            for p in range(NP):
                logits_pair("l", p, 0, kT)

            # --- remote k, v from the partner's hT (gathered) ---
            for m in range(NP):
                ps = psum_mm.tile([128, 512], F32, tag="mm")
                for k in range(KC):
                    nc.tensor.matmul(
                        ps[:], wqk_sb[:, k, 512 + m * 128:512 + (m + 1) * 128],
                        hT_rem[:, k, :], start=(k == 0), stop=(k == KC - 1))
                dst = kT[:, m, NB:2 * NB, :].rearrange("p a b -> p (a b)")
                evac2(dst, ps[:], bqk_sb[:, 4 + m:5 + m])
            for j in range(NB):
                ps = psum_mm.tile([128, 512], F32, tag="mm")
                for k in range(KC):
                    nc.tensor.matmul(
                        ps[:], hT_rem[:, k, j * 128:(j + 1) * 128], wv_sb[:, k, :],
                        start=(k == 0), stop=(k == KC - 1))
                nc.vector.tensor_copy(
                    v_all[:, NB + j, :, 0:D],
                    ps[:].rearrange("p (h d) -> p h d", h=H))

            # --- remote logits (kT slots 4-7) ---
            for p in range(NP):
                logits_pair("r", p, NB, kT)

            # --- PV (token-major, ones-column denominator), j-outer so each
            # chunk's normalize/LN1/transpose overlaps the remaining PV ---
            y1 = ypool.tile([128, NB, DM], F16, tag="y")
            g1b = g1_sb[:].rearrange("p d -> p () d").broadcast_to([128, 1, DM])
            bobb = bob_sb[:].rearrange("p d -> p () d").broadcast_to([128, 1, DM])
            for j in range(NB):
                for p in range(NP):
                    pv = psum_pv.tile([128, 2, D + 1], F32, tag="pv")
                    steps = ([("l", s) for s in range(j + 1)]
                             + [("r", s) for s in range(j + 1)])
                    nstep = len(steps)
                    for hh in range(2):
                        for i, (which, s) in enumerate(steps):
                            slot = s if which == "l" else NB + s
                            pt, base = pts[(which, p, s)]
                            off = base + (j - s) * 128
                            nc.tensor.matmul(
                                pv[:, hh, :],
                                pt[:, hh, off:off + 128],
                                v_all[:, slot, 2 * p + hh, :],
                                start=(i == 0), stop=(i == nstep - 1))
                    nc.vector.tensor_copy(a_sb[:, j, 2 * p:2 * p + 2, :], pv[:])
                # normalize chunk j + residual add + LN1 + transpose
                nc.vector.reciprocal(rden[:, j, :], a_sb[:, j, :, D])
                nc.vector.tensor_mul(
                    a_sb[:, j, :, 0:D], a_sb[:, j, :, 0:D],
                    rden[:, j, :].rearrange("p b -> p b ()")
                    .broadcast_to([128, H, D]))
                nc.vector.tensor_add(
                    h[:, j, :].rearrange("p (b d) -> p b d", b=H),
                    h[:, j, :].rearrange("p (b d) -> p b d", b=H),
                    a_sb[:, j, :, 0:D])
                _ln_chunk(nc, ppool, h, y1, j, eps_sb)
                # x2base = y1*g1 + (b1+bo) on gpsimd, off critical path
                nc.gpsimd.tensor_mul(h[:, j:j + 1, :], y1[:, j:j + 1, :], g1b)
                nc.gpsimd.tensor_add(h[:, j:j + 1, :], h[:, j:j + 1, :], bobb)

            # --- FFN1 (feature-major), token-halves: half 0 runs while the
            # last chunk's LN chain + transpose are still finishing ---
            ff_sb = apool.tile([128, FC, LT], F16, tag="ff")

            def ffn1_half(half):
                cols = slice(half * 256, half * 256 + 256)
                for m in range(FC):
                    ps = psum_mm.tile([128, 256], F32, tag="mm")
                    for k in range(KC):
                        nc.tensor.matmul(
                            ps[:], wff_sb[:, k, m * 128:(m + 1) * 128],
                            h1T[:, k, cols], start=(k == 0), stop=(k == KC - 1))
                    dst = ff_sb[:, m, cols]
                    if m % 2 == 0:
                        nc.scalar.activation(dst, ps[:], AF.Relu,
                                             bias=bff_sb[:, m:m + 1])
                    else:
                        nc.vector.tensor_scalar(dst, ps[:], bff_sb[:, m:m + 1],
                                                0.0, mybir.AluOpType.add,
                                                mybir.AluOpType.max)

            # transposes out of the PV loop: each waits its chunk's vector
            # LN chain, so inline placement would stall the PE between PV
            # groups; here they pipeline against FFN1
            transpose_chunk(h1T, y1, 0)
            transpose_chunk(h1T, y1, 1)
            ffn1_half(0)
            transpose_chunk(h1T, y1, 2)
            transpose_chunk(h1T, y1, 3)
            ffn1_half(1)

            # --- FFN2 + residual + LN2; PE-transpose y2 -> hT; exchange ---
            y2 = ypool.tile([128, NB, DM], F16, tag="y")
            for j in range(NB):
                ps = psum_mm.tile([128, DM], F32, tag="mm")
                for k in range(FC):
                    nc.tensor.matmul(
                        ps[:], ff_sb[:, k, j * 128:(j + 1) * 128], wo_sb[:, k, :],
                        start=(k == 0), stop=(k == FC - 1))
                nc.vector.tensor_add(h[:, j, :], h[:, j, :], ps[:])
                _ln_chunk(nc, ppool, h, y2, j, eps_sb)
                if l < L - 1 and j == 0:
                    wnext = load_weights(l + 1, nc.sync)
            if l < L - 1:
                for j in range(NB):
                    transpose_chunk(hT, y2, j)
                # next-layer residual base: h = y2*g2[l] + cvec[l+1] — on
                # vector so the gpsimd queue goes straight to the collective
                g2b = g2_sb[:].rearrange("p d -> p () d").broadcast_to([128, NB, DM])
                cvbn = wnext["cvec"][:].rearrange("p d -> p () d").broadcast_to(
                    [128, NB, DM])
                nc.vector.tensor_mul(h[:], y2[:], g2b)
                nc.vector.tensor_add(h[:], h[:], cvbn)
                ago_infl = hT_exchange()
            trsrc = y2

        # --- output: y2*g2[L-1] + b2[L-1], cast to f32 ---
        b2f_sb = singles.tile([128, DM], F16)
        nc.sync.dma_start(b2f_sb[:], b2f_rep[:])
        g2f_sb = w["g2"]
        for j in range(NB):
            nc.vector.tensor_mul(h[:, j, :], trsrc[:, j, :], g2f_sb[:])
            nc.vector.tensor_add(h[:, j, :], h[:, j, :], b2f_sb[:])
            nc.scalar.copy(ho[:, j, :], h[:, j, :])
            nc.sync.dma_start(out[:, j, :], ho[:, j, :])

    nc.finalize()
    return nc


def _ln_chunk(nc, pool, x, y, t, eps_sb):
    """LayerNorm (no affine) of chunk t: y[:, t, :] = (x_t - mean)/std."""
    stats = pool.tile([128, NB, 6], F32, tag="ln_stats")
    mv = pool.tile([128, NB, 2], F32, tag="ln_mv")
    rstd = pool.tile([128, NB], F32, tag="ln_rstd")
    nc.vector.bn_stats(stats[:, t, :], x[:, t, :])
    nc.vector.bn_aggr(mv[:, t, :], stats[:, t, :])
    nc.scalar.activation(rstd[:, t:t + 1], mv[:, t, 1:2], AF.Sqrt, bias=eps_sb[:])
    nc.vector.reciprocal(rstd[:, t:t + 1], rstd[:, t:t + 1])
    nc.vector.tensor_scalar(
        y[:, t, :], x[:, t, :],
        mv[:, t, 0:1], rstd[:, t:t + 1],
        mybir.AluOpType.subtract, mybir.AluOpType.mult,
    )


_NC_CACHE = {}


def _get_nc(**kw):
    key = tuple(sorted(kw.items()))
    if key not in _NC_CACHE:
        _NC_CACHE[key] = _build(**kw)
    return _NC_CACHE[key]


def _prep_inputs(x, emb, Wqkv, bqkv, Wff, bff, Wo, bo, g1, beta1, g2, beta2):
    """Host-side sharding + LN-affine folding: build the 8 per-core maps."""
    f16 = np.float16
    f32 = np.float32
    emb16 = np.ascontiguousarray((np.asarray(emb) * np.sqrt(f32(DM))).astype(f16))

    p_ = np.arange(T, dtype=f32)[:, None]
    i_ = np.arange(DM, dtype=f32)[None, :]
    rates = 1.0 / np.power(10000.0, 2.0 * np.floor(i_ / 2.0) / DM)
    ang = p_ * rates
    even = (np.arange(DM) % 2) == 0
    pos_full = np.where(even[None, :], np.sin(ang), np.cos(ang)).astype(f16)

    Wqkv = np.asarray(Wqkv, f32)
    bqkv = np.asarray(bqkv, f32)
    Wff_ = np.asarray(Wff, f32)
    Wo_ = np.asarray(Wo, f32)
    bff_ = np.asarray(bff, f32)
    bo_ = np.asarray(bo, f32)
    g1_ = np.asarray(g1, f32)
    b1_ = np.asarray(beta1, f32)
    g2_ = np.asarray(g2, f32)
    b2_ = np.asarray(beta2, f32)

    # fold g2[l-1], b2[l-1] into layer l's QKV weights (l >= 1)
    gprev = np.ones((L, DM), f32)
    bprev = np.zeros((L, DM), f32)
    gprev[1:] = g2_[:-1]
    bprev[1:] = b2_[:-1]
    Wqkv_f = Wqkv * gprev[:, :, None]
    bqkv_f = bqkv + np.einsum("ld,ldc->lc", bprev, Wqkv)
    # fold g1[l], b1[l] into Wff
    Wff_f = Wff_ * g1_[:, :, None]
    bff_f = bff_ + np.einsum("ld,ldc->lc", b1_, Wff_)
    bob = (bo_ + b1_).astype(f16)

    Wh = Wqkv_f.reshape(L, DM, H, D, 3)
    bh = bqkv_f.reshape(L, H, D, 3)

    def dm_part(w):  # [L, DM, C] -> [L, 128, KC, C]
        Lx, dm, C = w.shape
        return np.ascontiguousarray(
            w.reshape(Lx, dm // 128, 128, C).transpose(0, 2, 1, 3))

    wq = Wh[:, :, :, :, 0].reshape(L, DM, DM)
    wk = Wh[:, :, :, :, 1].reshape(L, DM, DM)
    wqk_l = dm_part(np.concatenate([wq, wk], axis=2)).astype(f16)
    bq = bh[:, :, :, 0].reshape(L, DM)
    bk = bh[:, :, :, 1].reshape(L, DM)
    bqk_l = np.ascontiguousarray(np.concatenate(
        [bq.reshape(L, 4, 128), bk.reshape(L, 4, 128)],
        axis=1).transpose(0, 2, 1)).astype(f32)
    wv_l = dm_part(Wh[:, :, :, :, 2].reshape(L, DM, DM)).astype(f16)

    wff_l = dm_part(Wff_f).astype(f16)
    wo_l = dm_part(Wo_).astype(f16)
    bff_l = np.ascontiguousarray(bff_f.reshape(L, FC, 128).transpose(0, 2, 1))

    def rep(v):  # [L, DM] -> [L, 128, DM] replicated f16
        return np.ascontiguousarray(np.broadcast_to(
            np.asarray(v, f16)[:, None, :], (L, 128, DM)))

    bv_full = bh[:, :, :, 2].reshape(L, DM)
    cvec = bv_full + bprev

    bob_l = rep(bob)
    cvec_l = rep(cvec)
    g1_l = rep(g1_)
    g2_l = rep(g2_)
    b2f = np.ascontiguousarray(
        np.broadcast_to(b2_[L - 1].astype(f16)[None, :], (128, DM)))

    s_i = np.arange(128)[:, None]
    q_i = np.arange(128)[None, :]
    diag = (s_i <= q_i).astype(f16)
    identm = np.eye(128, dtype=f16)

    def wrap_idx(ids):  # [n] -> [128, n//16] int16
        n = len(ids)
        return np.ascontiguousarray(
            np.tile(np.asarray(ids, np.int64).reshape(n // 16, 16).T
                    .astype(np.int16), (8, 1)))

    x = np.asarray(x)
    in_maps = []
    for c in range(8):
        seq, par = c // 2, c % 2
        B = P0 if par == 0 else P1
        R = P1 if par == 0 else P0

        ids = np.concatenate([np.asarray(x[seq][b * 128:(b + 1) * 128], np.int64)
                              for b in B])
        idx_w = wrap_idx(ids)
        idxr_w = wrap_idx((1 - par) * 128 + np.arange(128))

        pos_l = np.ascontiguousarray(np.stack(
            [pos_full[b * 128:(b + 1) * 128] for b in B], axis=1))  # [128,NB,DM]

        mr = np.zeros((128, NB), f16)
        for s in range(NB):
            mr[:, s] = 1.0 if R[s] < B[s] else 0.0

        in_maps.append({
            "emb16": emb16, "idxs": idx_w, "idxr": idxr_w, "pos": pos_l,
            "wqk": wqk_l, "bqk": bqk_l, "wv": wv_l,
            "wff": wff_l, "bff": bff_l, "wo": wo_l, "bob_rep": bob_l,
            "cvec_rep": cvec_l, "g1_rep": g1_l, "g2_rep": g2_l, "b2f_rep": b2f,
            "diagm": diag, "maskr": np.ascontiguousarray(mr), "ident": identm,
        })
    return in_maps


def kernel(**inputs) -> np.ndarray:
    nc = _get_nc()
    in_maps = _prep_inputs(**inputs)
    res = run_bass_kernel_spmd(nc, in_maps, core_ids=list(range(8)))
    full = np.zeros((N, T, DM), np.float32)
    for c in range(8):
        o = res.results[c]["out"]  # [128, NB, DM]
        B = P0 if c % 2 == 0 else P1
        for jj, b in enumerate(B):
            full[c // 2, b * 128:(b + 1) * 128, :] = o[:, jj, :]
    return full


# revision 4
# speedup vs baseline: 1.0636x; 1.0034x over previous
"""Trainium2 Bass kernel v2 for nn_Decoder_78237124264042.

Query-split design: 8 cores = 4 sequence-pairs. Pair {2i, 2i+1} handles
sequence i; parity-0 core owns token blocks {0,3,4,7}, parity-1 {1,2,5,6}
(causally balanced). Each core runs attention for ALL 8 heads over its 4
local query blocks and keeps LN/FFN local (half-size tokens).

Cross-core data is ONE 0.5MB AllGather of the feature-major y2 (hT) per
layer, issued right after the FFN2 transposes and consumed a full
QKV+local-attention window later (~25us of cover vs ~15-20us collective
latency incl. the peer-driven inbound lag) — so it never stalls. The
rank-relative readback uses dma_gather with per-core index data (rows of
the partner's AllGather half), keeping the program SPMD-uniform. Remote
tokens' k/v are then computed locally from hT_rem (the extra PE work fits
inside the scalar-bound local-exp window), which makes the kT/v tiles
fully uniform: slots 0-3 local blocks, 4-7 remote blocks.

Causal structure (uniform across parity):
- local key step s vs local query ordinal j: attend iff s <= j, diagonal
  mask exactly at s == j (compile-time uniform).
- remote key step s vs query ordinal j: superset s <= j; the only blocks
  that differ by parity are s == j, handled by a 0/1 per-(core,s) mask
  input applied to the first 128 columns of the remote pT tile.

Head-pair packing: q is stored in qe/qo tiles with zeroed partition
halves so one kT stationary serves both heads of a pair; exp and mask
run once per (pair, step). PV output is token-major with a ones-column
denominator; normalization+residual+LN1+PE-transpose run per query chunk
(j-outer PV) so they overlap the remaining PV. All transposes are PE
transposes (identity matmul into PSUM) — no DRAM bounce anywhere.
"""
import numpy as np
from contextlib import ExitStack

import concourse.bass as bass
import concourse.tile as tile
from concourse import bacc, mybir
from concourse.bass_utils import run_bass_kernel_spmd

V, L, H, D, DFF = 32000, 6, 8, 64, 1024
DM = H * D  # 512
N, T = 4, 1024
EPS = 1e-3
NB = 4               # local 128-token blocks per core
LT = NB * 128        # local tokens = 512
KC = DM // 128       # 4
FC = DFF // 128      # 8
NP = H // 2          # head pairs = 4
SCALE = 1.0 / np.sqrt(D)
F16 = mybir.dt.float16
F32 = mybir.dt.float32
AF = mybir.ActivationFunctionType

P0 = [0, 3, 4, 7]
P1 = [1, 2, 5, 6]

HTW = KC * LT        # hT payload cols = 2048 (4096B row, 256B-aligned)


def _build(debug=False, no_cc=False):
    nc = bacc.Bacc(
        "TRN2",
        target_bir_lowering=False,
        debug=False,
        enable_asserts=True,
        num_devices=8,
    )

    def din(name, shape, dt=F16):
        return nc.dram_tensor(name, shape, dt, kind="ExternalInput").ap()

    emb = din("emb16", [V, DM])
    idxs = din("idxs", [128, LT // 16], mybir.dt.int16)
    idxr = din("idxr", [128, 128 // 16], mybir.dt.int16)
    pos = din("pos", [128, NB, DM])
    wqk = din("wqk", [L, 128, KC, 1024])      # [q(512) | k(512)], g2[l-1]-folded
    bqk = din("bqk", [L, 128, 8], F32)        # q psum m -> col m, k -> col 4+m
    wv = din("wv", [L, 128, KC, DM])          # g2[l-1]-folded
    wff = din("wff", [L, 128, KC, DFF])       # g1[l]-folded
    bff = din("bff", [L, 128, FC], F32)
    wo = din("wo", [L, 128, FC, DM])
    bob_rep = din("bob_rep", [L, 128, DM])    # bo + b1
    cvec_rep = din("cvec_rep", [L, 128, DM])  # bv_full + b2[l-1]
    g1_rep = din("g1_rep", [L, 128, DM])
    g2_rep = din("g2_rep", [L, 128, DM])
    b2f_rep = din("b2f_rep", [128, DM])
    diagm = din("diagm", [128, 128])          # keep-mask (key<=query)
    maskr = din("maskr", [128, NB])           # 0/1 remote diag-step mask
    ident = din("ident", [128, 128])

    out = nc.dram_tensor("out", [128, NB, DM], F32, kind="ExternalOutput").ap()

    def prb(name, tile_ap):
        if not debug:
            return
        t = nc.dram_tensor(f"prb_{name}", list(tile_ap.shape), tile_ap.dtype,
                           kind="ExternalOutput").ap()
        nc.sync.dma_start(t[:], tile_ap)

    with tile.TileContext(nc) as tc, ExitStack() as ctx:
        singles = ctx.enter_context(tc.tile_pool(name="singles", bufs=1))
        wpool = ctx.enter_context(tc.tile_pool(name="wpool", bufs=2))
        ypool = ctx.enter_context(tc.tile_pool(name="ypool", bufs=2))
        ppool = ctx.enter_context(tc.tile_pool(name="ppool", bufs=3))
        ptpool = ctx.enter_context(tc.tile_pool(name="ptpool", bufs=1))
        apool = ctx.enter_context(tc.tile_pool(name="apool", bufs=1))
        psum_lg = ctx.enter_context(tc.tile_pool(name="psum_lg", bufs=2, space="PSUM"))
        psum_mm = ctx.enter_context(tc.tile_pool(name="psum_mm", bufs=2, space="PSUM"))
        psum_pv = ctx.enter_context(tc.tile_pool(name="psum_pv", bufs=2, space="PSUM"))
        dram = ctx.enter_context(tc.tile_pool(name="dram", bufs=2, space="DRAM"))

        # --- persistent tiles ---
        h = singles.tile([128, NB, DM], F16)      # token-major residual (local)
        hT = singles.tile([128, KC, LT], F16)     # feature-major y2 (local)
        hTr_g = singles.tile([128, 1, HTW], F16)  # gather dst: partner's hT
        h1T = singles.tile([128, KC, LT], F16)    # feature-major y1 (FFN input)
        qe = singles.tile([128, NP, LT], F16)     # even head q; rows 64-127 zero
        qo = singles.tile([128, NP, LT], F16)     # odd head q; rows 0-63 zero
        kT = singles.tile([128, NP, 2 * NB, 128], F16)   # slots: 0-3 loc, 4-7 rem
        v_all = singles.tile([128, 2 * NB, H, D + 1], F16)
        a_sb = singles.tile([128, NB, H, D + 1], F16)
        rden = singles.tile([128, NB, H], F32)
        pos_sb = singles.tile([128, NB, DM], F16)
        idxs_sb = singles.tile([128, LT // 16], mybir.dt.int16)
        idxr_sb = singles.tile([128, 128 // 16], mybir.dt.int16)
        diag_sb = singles.tile([128, 128], F16)
        maskr_sb = singles.tile([128, NB], F16)
        ident_sb = singles.tile([128, 128], F16)
        eps_sb = singles.tile([128, 1], F32)
        ho = singles.tile([128, NB, DM], F32)

        hT_rem = hTr_g[:, 0, :].rearrange("p (a b) -> p a b", a=KC, b=LT)

        nc.vector.memset(eps_sb[:], EPS)
        nc.vector.memset(qe[64:128, :, :], 0.0)
        nc.vector.memset(qo[0:64, :, :], 0.0)
        nc.vector.memset(v_all[:, :, :, D:D + 1], 1.0)
        nc.sync.dma_start(pos_sb[:], pos[:])
        idx_load = nc.sync.dma_start(idxs_sb[:], idxs[:])
        idxr_load = nc.sync.dma_start(idxr_sb[:], idxr[:])
        nc.sync.dma_start(diag_sb[:], diagm[:])
        nc.sync.dma_start(maskr_sb[:], maskr[:])
        nc.sync.dma_start(ident_sb[:], ident[:])

        # warm-up collective: absorbs first-collective rendezvous cost.
        wua = dram.tile([128, 16], F16, tag="wua")
        wuo = dram.tile([256, 16], F16, tag="wuo")
        nc.sync.dma_start(wua[:], diag_sb[:, 0:16])
        if not no_cc:
            nc.gpsimd.collective_compute(
                "AllGather", mybir.AluOpType.bypass,
                replica_groups=[[0, 1], [2, 3], [4, 5], [6, 7]],
                ins=[wua[:].opt()], outs=[wuo[:].opt()],
            )

        # --- embedding gather for local tokens ---
        gat = nc.gpsimd.dma_gather(
            h[:], emb[:], idxs_sb[:],
            num_idxs=LT, num_idxs_reg=LT, elem_size=DM, elem_step=DM,
        )
        tile.add_dep_helper(gat.ins, idx_load.ins, reason="gather reads idxs_sb")
        nc.vector.tensor_add(h[:], h[:], pos_sb[:])
        prb("h0", h[:])

        def load_weights(l, eng):
            shapes = {
                "wqk": ([128, KC, 1024], F16), "wv": ([128, KC, DM], F16),
                "wff": ([128, KC, DFF], F16), "wo": ([128, FC, DM], F16),
                "bqk": ([128, 8], F32), "bff": ([128, FC], F32),
                "bob": ([128, DM], F16), "cvec": ([128, DM], F16),
                "g1": ([128, DM], F16), "g2": ([128, DM], F16),
            }
            w = {k: wpool.tile(s, dt, tag=k, name=f"w_{k}")
                 for k, (s, dt) in shapes.items()}
            for name, src_t in [("wqk", wqk), ("wv", wv), ("wff", wff),
                                ("wo", wo), ("bqk", bqk), ("bff", bff)]:
                eng.dma_start(w[name][:], src_t[l])
            eng.dma_start(w["bob"][:], bob_rep[l])
            eng.dma_start(w["cvec"][:], cvec_rep[l])
            eng.dma_start(w["g1"][:], g1_rep[l])
            eng.dma_start(w["g2"][:], g2_rep[l])
            return w

        def transpose_chunk(dst, src, j, eng=None):
            """PE-transpose src[:, j, :] ([128 tok, DM]) into dst[:, :, j*128:...]."""
            tr = psum_mm.tile([128, KC, 128], F16, tag="mm", name="tr")
            for k in range(KC):
                nc.tensor.transpose(
                    tr[:, k, :], src[:, j, k * 128:(k + 1) * 128], ident_sb[:])
            if eng is None:
                nc.scalar.copy(dst[:, :, j * 128:(j + 1) * 128], tr[:])
            else:
                eng.tensor_copy(dst[:, :, j * 128:(j + 1) * 128], tr[:])

        def hT_exchange():
            """AllGather local hT; partner's half lands in hTr_g via gather."""
            agi = dram.tile([128, HTW], F16, tag="agi")
            ago = dram.tile([256, HTW], F16, tag="ago")
            nc.sync.dma_start(agi[:], hT[:].rearrange("p a b -> p (a b)"))
            if no_cc:
                nc.sync.dma_start(ago[0:128], agi[:])
                nc.sync.dma_start(ago[128:256], agi[:])
            else:
                nc.gpsimd.collective_compute(
                    "AllGather", mybir.AluOpType.bypass,
                    replica_groups=[[0, 1], [2, 3], [4, 5], [6, 7]],
                    ins=[agi[:].opt()], outs=[ago[:].opt()],
                )
            return ago

        def hT_collect(ago):
            g = nc.gpsimd.dma_gather(
                hTr_g[:], ago[:], idxr_sb[:],
                num_idxs=128, num_idxs_reg=128, elem_size=HTW, elem_step=HTW,
            )
            tile.add_dep_helper(g.ins, idxr_load.ins, reason="hT gather idxr")

        # layer 0 feature-major input + first exchange
        wnext = load_weights(0, nc.sync)
        for j in range(NB):
            transpose_chunk(hT, h, j, eng=nc.vector)
        ago_infl = hT_exchange()
        # layer-0 residual base: h += cvec[0] (after the exchange so the
        # gpsimd queue reaches the collective without a 4us detour)
        cvb0 = wnext["cvec"][:].rearrange("p d -> p () d").broadcast_to([128, NB, DM])
        nc.gpsimd.tensor_add(h[:], h[:], cvb0)

        for l in range(L):
            w = wnext
            wqk_sb, wv_sb, wff_sb, wo_sb = w["wqk"], w["wv"], w["wff"], w["wo"]
            bqk_sb, bff_sb, bob_sb = w["bqk"], w["bff"], w["bob"]
            g1_sb, g2_sb = w["g1"], w["g2"]

            # collect partner's hT (issued at the end of the previous layer)
            hT_collect(ago_infl)

            # psum evacuation split across scalar+vector so the mm ring
            # frees in ~half the time
            def evac2(dst, ps, bias):
                hw = ps.shape[-1] // 2
                nc.scalar.activation(dst[:, 0:hw], ps[:, 0:hw], AF.Identity,
                                     bias=bias)
                nc.vector.tensor_scalar(dst[:, hw:], ps[:, hw:], bias,
                                        None, mybir.AluOpType.add)

            # pts[(which, p, s)] = (tile, col base of step s within it);
            # steps 2 and 3 share one psum tile + one exp.
            pts = {}

            def pt_mask(which, pt, s, base):
                if which == "l":
                    nc.vector.tensor_mul(
                        pt[:, :, base:base + 128], pt[:, :, base:base + 128],
                        diag_sb[:].rearrange("p d -> p () d")
                        .broadcast_to([128, 2, 128]))
                else:
                    nc.vector.tensor_mul(
                        pt[:, :, base:base + 128], pt[:, :, base:base + 128],
                        maskr_sb[:, s:s + 1].rearrange("p o -> p o ()")
                        .broadcast_to([128, 2, 128]))

            def logits_block(which, p, s, slot_base, kt_tile):
                C = (NB - s) * 128
                lg = psum_lg.tile([128, 2, 512], F32, tag="lg")
                nc.tensor.matmul(lg[:, 0, 0:C], kt_tile[:, p, slot_base + s, :],
                                 qe[:, p, s * 128:LT], start=True, stop=True)
                nc.tensor.matmul(lg[:, 1, 0:C], kt_tile[:, p, slot_base + s, :],
                                 qo[:, p, s * 128:LT], start=True, stop=True)
                pt = ptpool.tile([128, 2, C], F16,
                                 tag=f"pt{which}{p}{s}", name="pt")
                nc.scalar.activation(pt[:], lg[:, :, 0:C], AF.Exp,
                                     scale=float(SCALE))
                pt_mask(which, pt, s, 0)
                pts[(which, p, s)] = (pt, 0)

            def logits_block23(which, p, slot_base, kt_tile):
                lg = psum_lg.tile([128, 2, 512], F32, tag="lg")
                for hh, q_t in ((0, qe), (1, qo)):
                    nc.tensor.matmul(lg[:, hh, 0:256],
                                     kt_tile[:, p, slot_base + 2, :],
                                     q_t[:, p, 256:LT], start=True, stop=True)
                    nc.tensor.matmul(lg[:, hh, 256:384],
                                     kt_tile[:, p, slot_base + 3, :],
                                     q_t[:, p, 384:LT], start=True, stop=True)
                pt = ptpool.tile([128, 2, 384], F16,
                                 tag=f"pt{which}{p}23", name="pt")
                nc.scalar.activation(pt[:], lg[:, :, 0:384], AF.Exp,
                                     scale=float(SCALE))
                pt_mask(which, pt, 2, 0)
                pt_mask(which, pt, 3, 256)
                pts[(which, p, 2)] = (pt, 0)
                pts[(which, p, 3)] = (pt, 256)

            def logits_pair(which, p, slot_base, kt_tile):
                logits_block(which, p, 0, slot_base, kt_tile)
                logits_block(which, p, 1, slot_base, kt_tile)
                logits_block23(which, p, slot_base, kt_tile)

            # --- local k, then q interleaved with that pair's local logits
            # (exp stream starts ~4us earlier; v psums fill the PE behind
            # the exp-gated logits groups) ---
            for m in range(NP):
                ps = psum_mm.tile([128, 512], F32, tag="mm")
                for k in range(KC):
                    nc.tensor.matmul(
                        ps[:], wqk_sb[:, k, 512 + m * 128:512 + (m + 1) * 128],
                        hT[:, k, :], start=(k == 0), stop=(k == KC - 1))
                dst = kT[:, m, 0:NB, :].rearrange("p a b -> p (a b)")
                evac2(dst, ps[:], bqk_sb[:, 4 + m:5 + m])
            for m in range(NP):
                ps = psum_mm.tile([128, 512], F32, tag="mm")
                for k in range(KC):
                    nc.tensor.matmul(
                        ps[:], wqk_sb[:, k, m * 128:(m + 1) * 128],
                        hT[:, k, :], start=(k == 0), stop=(k == KC - 1))
                nc.scalar.activation(qe[0:64, m, 0:256], ps[0:64, 0:256],
                                     AF.Identity, bias=bqk_sb[0:64, m:m + 1])
                nc.vector.tensor_scalar(qe[0:64, m, 256:512], ps[0:64, 256:512],
                                        bqk_sb[0:64, m:m + 1], None,
                                        mybir.AluOpType.add)
                nc.scalar.activation(qo[64:128, m, 0:256], ps[64:128, 0:256],
                                     AF.Identity, bias=bqk_sb[64:128, m:m + 1])
                nc.vector.tensor_scalar(qo[64:128, m, 256:512],
                                        ps[64:128, 256:512],
                                        bqk_sb[64:128, m:m + 1], None,
                                        mybir.AluOpType.add)
                logits_pair("l", m, 0, kT)
            for j in range(NB):
                ps = psum_mm.tile([128, 512], F32, tag="mm")
                for k in range(KC):
                    nc.tensor.matmul(
                        ps[:], hT[:, k, j * 128:(j + 1) * 128], wv_sb[:, k, :],
                        start=(k == 0), stop=(k == KC - 1))
                nc.vector.tensor_copy(
                    v_all[:, j, :, 0:D],
                    ps[:].rearrange("p (h d) -> p h d", h=H))

            # --- remote k, v from the partner's hT (gathered) ---
            for m in range(NP):
                ps = psum_mm.tile([128, 512], F32, tag="mm")
                for k in range(KC):
                    nc.tensor.matmul(
                        ps[:], wqk_sb[:, k, 512 + m * 128:512 + (m + 1) * 128],
                        hT_rem[:, k, :], start=(k == 0), stop=(k == KC - 1))
                dst = kT[:, m, NB:2 * NB, :].rearrange("p a b -> p (a b)")
                evac2(dst, ps[:], bqk_sb[:, 4 + m:5 + m])
            for j in range(NB):
                ps = psum_mm.tile([128, 512], F32, tag="mm")
                for k in range(KC):
                    nc.tensor.matmul(
                        ps[:], hT_rem[:, k, j * 128:(j + 1) * 128], wv_sb[:, k, :],
                        start=(k == 0), stop=(k == KC - 1))
                nc.vector.tensor_copy(
                    v_all[:, NB + j, :, 0:D],
                    ps[:].rearrange("p (h d) -> p h d", h=H))

            # --- remote logits (kT slots 4-7) ---
            for p in range(NP):
                logits_pair("r", p, NB, kT)

            # --- PV (token-major, ones-column denominator), j-outer so each
            # chunk's normalize/LN1/transpose overlaps the remaining PV ---
            y1 = ypool.tile([128, NB, DM], F16, tag="y")
            g1b = g1_sb[:].rearrange("p d -> p () d").broadcast_to([128, 1, DM])
            bobb = bob_sb[:].rearrange("p d -> p () d").broadcast_to([128, 1, DM])
            for j in range(NB):
                for p in range(NP):
                    pv = psum_pv.tile([128, 2, D + 1], F32, tag="pv")
                    steps = ([("l", s) for s in range(j + 1)]
                             + [("r", s) for s in range(j + 1)])
                    nstep = len(steps)
                    for hh in range(2):
                        for i, (which, s) in enumerate(steps):
                            slot = s if which == "l" else NB + s
                            pt, base = pts[(which, p, s)]
                            off = base + (j - s) * 128
                            nc.tensor.matmul(
                                pv[:, hh, :],
                                pt[:, hh, off:off + 128],
                                v_all[:, slot, 2 * p + hh, :],
                                start=(i == 0), stop=(i == nstep - 1))
                    nc.vector.tensor_copy(a_sb[:, j, 2 * p:2 * p + 2, :], pv[:])
                # normalize chunk j + residual add + LN1 + transpose
                nc.vector.reciprocal(rden[:, j, :], a_sb[:, j, :, D])
                nc.vector.tensor_mul(
                    a_sb[:, j, :, 0:D], a_sb[:, j, :, 0:D],
                    rden[:, j, :].rearrange("p b -> p b ()")
                    .broadcast_to([128, H, D]))
                nc.vector.tensor_add(
                    h[:, j, :].rearrange("p (b d) -> p b d", b=H),
                    h[:, j, :].rearrange("p (b d) -> p b d", b=H),
                    a_sb[:, j, :, 0:D])
                _ln_chunk(nc, ppool, h, y1, j, eps_sb)
                # x2base = y1*g1 + (b1+bo) on gpsimd, off critical path
                nc.gpsimd.tensor_mul(h[:, j:j + 1, :], y1[:, j:j + 1, :], g1b)
                nc.gpsimd.tensor_add(h[:, j:j + 1, :], h[:, j:j + 1, :], bobb)

            # --- FFN1 (feature-major), token-halves: half 0 runs while the
            # last chunk's LN chain + transpose are still finishing ---
            ff_sb = apool.tile([128, FC, LT], F16, tag="ff")

            def ffn1_half(half):
                cols = slice(half * 256, half * 256 + 256)
                for m in range(FC):
                    ps = psum_mm.tile([128, 256], F32, tag="mm")
                    for k in range(KC):
                        nc.tensor.matmul(
                            ps[:], wff_sb[:, k, m * 128:(m + 1) * 128],
                            h1T[:, k, cols], start=(k == 0), stop=(k == KC - 1))
                    dst = ff_sb[:, m, cols]
                    if m % 2 == 0:
                        nc.scalar.activation(dst, ps[:], AF.Relu,
                                             bias=bff_sb[:, m:m + 1])
                    else:
                        nc.vector.tensor_scalar(dst, ps[:], bff_sb[:, m:m + 1],
                                                0.0, mybir.AluOpType.add,
                                                mybir.AluOpType.max)

            # transposes out of the PV loop: each waits its chunk's vector
            # LN chain, so inline placement would stall the PE between PV
            # groups; here they pipeline against FFN1
            transpose_chunk(h1T, y1, 0)
            transpose_chunk(h1T, y1, 1)
            ffn1_half(0)
            transpose_chunk(h1T, y1, 2)
            transpose_chunk(h1T, y1, 3)
            ffn1_half(1)

            # --- FFN2 + residual + LN2; PE-transpose y2 -> hT; exchange ---
            y2 = ypool.tile([128, NB, DM], F16, tag="y")
            for j in range(NB):
                ps = psum_mm.tile([128, DM], F32, tag="mm")
                for k in range(FC):
                    nc.tensor.matmul(
                        ps[:], ff_sb[:, k, j * 128:(j + 1) * 128], wo_sb[:, k, :],
                        start=(k == 0), stop=(k == FC - 1))
                nc.vector.tensor_add(h[:, j, :], h[:, j, :], ps[:])
                _ln_chunk(nc, ppool, h, y2, j, eps_sb)
                if l < L - 1 and j == 0:
                    wnext = load_weights(l + 1, nc.sync)
            if l < L - 1:
                for j in range(NB):
                    transpose_chunk(hT, y2, j)
                # next-layer residual base: h = y2*g2[l] + cvec[l+1] — on
                # vector so the gpsimd queue goes straight to the collective
                g2b = g2_sb[:].rearrange("p d -> p () d").broadcast_to([128, NB, DM])
                cvbn = wnext["cvec"][:].rearrange("p d -> p () d").broadcast_to(
                    [128, NB, DM])
                nc.vector.tensor_mul(h[:], y2[:], g2b)
                nc.vector.tensor_add(h[:], h[:], cvbn)
                ago_infl = hT_exchange()
            trsrc = y2

        # --- output: y2*g2[L-1] + b2[L-1], cast to f32 ---
        b2f_sb = singles.tile([128, DM], F16)
        nc.sync.dma_start(b2f_sb[:], b2f_rep[:])
        g2f_sb = w["g2"]
        for j in range(NB):
            nc.vector.tensor_mul(h[:, j, :], trsrc[:, j, :], g2f_sb[:])
            nc.vector.tensor_add(h[:, j, :], h[:, j, :], b2f_sb[:])
            nc.scalar.copy(ho[:, j, :], h[:, j, :])
            nc.sync.dma_start(out[:, j, :], ho[:, j, :])

    nc.finalize()
    return nc


def _ln_chunk(nc, pool, x, y, t, eps_sb):
    """LayerNorm (no affine) of chunk t: y[:, t, :] = (x_t - mean)/std."""
    stats = pool.tile([128, NB, 6], F32, tag="ln_stats")
    mv = pool.tile([128, NB, 2], F32, tag="ln_mv")
    rstd = pool.tile([128, NB], F32, tag="ln_rstd")
    nc.vector.bn_stats(stats[:, t, :], x[:, t, :])
    nc.vector.bn_aggr(mv[:, t, :], stats[:, t, :])
    nc.scalar.activation(rstd[:, t:t + 1], mv[:, t, 1:2], AF.Sqrt, bias=eps_sb[:])
    nc.vector.reciprocal(rstd[:, t:t + 1], rstd[:, t:t + 1])
    nc.vector.tensor_scalar(
        y[:, t, :], x[:, t, :],
        mv[:, t, 0:1], rstd[:, t:t + 1],
        mybir.AluOpType.subtract, mybir.AluOpType.mult,
    )


_NC_CACHE = {}


def _get_nc(**kw):
    key = tuple(sorted(kw.items()))
    if key not in _NC_CACHE:
        _NC_CACHE[key] = _build(**kw)
    return _NC_CACHE[key]


def _prep_inputs(x, emb, Wqkv, bqkv, Wff, bff, Wo, bo, g1, beta1, g2, beta2):
    """Host-side sharding + LN-affine folding: build the 8 per-core maps."""
    f16 = np.float16
    f32 = np.float32
    emb16 = np.ascontiguousarray((np.asarray(emb) * np.sqrt(f32(DM))).astype(f16))

    p_ = np.arange(T, dtype=f32)[:, None]
    i_ = np.arange(DM, dtype=f32)[None, :]
    rates = 1.0 / np.power(10000.0, 2.0 * np.floor(i_ / 2.0) / DM)
    ang = p_ * rates
    even = (np.arange(DM) % 2) == 0
    pos_full = np.where(even[None, :], np.sin(ang), np.cos(ang)).astype(f16)

    Wqkv = np.asarray(Wqkv, f32)
    bqkv = np.asarray(bqkv, f32)
    Wff_ = np.asarray(Wff, f32)
    Wo_ = np.asarray(Wo, f32)
    bff_ = np.asarray(bff, f32)
    bo_ = np.asarray(bo, f32)
    g1_ = np.asarray(g1, f32)
    b1_ = np.asarray(beta1, f32)
    g2_ = np.asarray(g2, f32)
    b2_ = np.asarray(beta2, f32)

    # fold g2[l-1], b2[l-1] into layer l's QKV weights (l >= 1)
    gprev = np.ones((L, DM), f32)
    bprev = np.zeros((L, DM), f32)
    gprev[1:] = g2_[:-1]
    bprev[1:] = b2_[:-1]
    Wqkv_f = Wqkv * gprev[:, :, None]
    bqkv_f = bqkv + np.einsum("ld,ldc->lc", bprev, Wqkv)
    # fold g1[l], b1[l] into Wff
    Wff_f = Wff_ * g1_[:, :, None]
    bff_f = bff_ + np.einsum("ld,ldc->lc", b1_, Wff_)
    bob = (bo_ + b1_).astype(f16)

    Wh = Wqkv_f.reshape(L, DM, H, D, 3)
    bh = bqkv_f.reshape(L, H, D, 3)

    def dm_part(w):  # [L, DM, C] -> [L, 128, KC, C]
        Lx, dm, C = w.shape
        return np.ascontiguousarray(
            w.reshape(Lx, dm // 128, 128, C).transpose(0, 2, 1, 3))

    wq = Wh[:, :, :, :, 0].reshape(L, DM, DM)
    wk = Wh[:, :, :, :, 1].reshape(L, DM, DM)
    wqk_l = dm_part(np.concatenate([wq, wk], axis=2)).astype(f16)
    bq = bh[:, :, :, 0].reshape(L, DM)
    bk = bh[:, :, :, 1].reshape(L, DM)
    bqk_l = np.ascontiguousarray(np.concatenate(
        [bq.reshape(L, 4, 128), bk.reshape(L, 4, 128)],
        axis=1).transpose(0, 2, 1)).astype(f32)
    wv_l = dm_part(Wh[:, :, :, :, 2].reshape(L, DM, DM)).astype(f16)

    wff_l = dm_part(Wff_f).astype(f16)
    wo_l = dm_part(Wo_).astype(f16)
    bff_l = np.ascontiguousarray(bff_f.reshape(L, FC, 128).transpose(0, 2, 1))

    def rep(v):  # [L, DM] -> [L, 128, DM] replicated f16
        return np.ascontiguousarray(np.broadcast_to(
            np.asarray(v, f16)[:, None, :], (L, 128, DM)))

    bv_full = bh[:, :, :, 2].reshape(L, DM)
    cvec = bv_full + bprev

    bob_l = rep(bob)
    cvec_l = rep(cvec)
    g1_l = rep(g1_)
    g2_l = rep(g2_)
    b2f = np.ascontiguousarray(
        np.broadcast_to(b2_[L - 1].astype(f16)[None, :], (128, DM)))

    s_i = np.arange(128)[:, None]
    q_i = np.arange(128)[None, :]
    diag = (s_i <= q_i).astype(f16)
    identm = np.eye(128, dtype=f16)

    def wrap_idx(ids):  # [n] -> [128, n//16] int16
        n = len(ids)
        return np.ascontiguousarray(
            np.tile(np.asarray(ids, np.int64).reshape(n // 16, 16).T
                    .astype(np.int16), (8, 1)))

    x = np.asarray(x)
    in_maps = []
    for c in range(8):
        seq, par = c // 2, c % 2
        B = P0 if par == 0 else P1
        R = P1 if par == 0 else P0

        ids = np.concatenate([np.asarray(x[seq][b * 128:(b + 1) * 128], np.int64)
                              for b in B])
        idx_w = wrap_idx(ids)
        idxr_w = wrap_idx((1 - par) * 128 + np.arange(128))

        pos_l = np.ascontiguousarray(np.stack(
            [pos_full[b * 128:(b + 1) * 128] for b in B], axis=1))  # [128,NB,DM]

        mr = np.zeros((128, NB), f16)
        for s in range(NB):
            mr[:, s] = 1.0 if R[s] < B[s] else 0.0

        in_maps.append({
            "emb16": emb16, "idxs": idx_w, "idxr": idxr_w, "pos": pos_l,
            "wqk": wqk_l, "bqk": bqk_l, "wv": wv_l,
            "wff": wff_l, "bff": bff_l, "wo": wo_l, "bob_rep": bob_l,
            "cvec_rep": cvec_l, "g1_rep": g1_l, "g2_rep": g2_l, "b2f_rep": b2f,
            "diagm": diag, "maskr": np.ascontiguousarray(mr), "ident": identm,
        })
    return in_maps


def kernel(**inputs) -> np.ndarray:
    nc = _get_nc()
    in_maps = _prep_inputs(**inputs)
    res = run_bass_kernel_spmd(nc, in_maps, core_ids=list(range(8)))
    full = np.zeros((N, T, DM), np.float32)
    for c in range(8):
        o = res.results[c]["out"]  # [128, NB, DM]
        B = P0 if c % 2 == 0 else P1
        for jj, b in enumerate(B):
            full[c // 2, b * 128:(b + 1) * 128, :] = o[:, jj, :]
    return full
